# revision 1
# baseline (speedup 1.0000x reference)
"""Trainium2 Bass kernel for nn_EncoderLayer_42399917146737.

The reference "SSM scan" is degenerate: at every step i the recurrence
overwrites h at exactly the positions p with pc[p,i]==1 with the scalar
b_i, and the step output reads only those positions.  Hence

    y_i[b] = C[b,i] * Bcoef[b,i] * n_i,      n_i = sum_p pc[p,i]

with no sequential dependence, and the reverse scan equals the forward
one.  The broadcast over p then reduces the Wr projection to a scalar
sum, so the whole module collapses to

    logits[b,l] = 2*sum(Wr) * has_err[b] * n_l * C[b,l] * (Bbias[b,l]/M + tanh(|X[b,l]|*wb_l))
    out         = softmax_l(logits)

where  Bbias = h0 @ pc,  h0 = 1-2*parity(hard @ pc^T),  hard = (X<0),
M = max|Bbias| (GLOBAL over the full batch),  wb = Wb @ pc,  wc = Wc @ pc,
C = 0.5 + tanh(|X|*wc_l).  (br shifts all logits equally -> drops out of
softmax.)

Sharding: batch B=128 over 8 cores (16 rows each).  Because M is a
global max over the whole batch, every core recomputes the (cheap)
full-batch parity/Bbias matmuls; the per-batch elementwise work + softmax
run only on the core's own 16 rows.  Per-core batch selection is done
with a per-core one-hot selection matrix (E_c) fed through the tensor
engine, so a single NEFF serves all 8 cores.

Precision: pc/hard/m are {0,1} so fp8/bf16 matmuls with f32 accumulate
are exact; X^T for sign tests rides in bf16 (sign-exact); Wb/Wc ride the
bf16 `pcl` matmul as hi+lo split columns (~2^-16 rel err); the local
elementwise path keeps full f32 X.
"""

import numpy as np
import ml_dtypes

B, L, P = 128, 1024, 512
NCORES = 8
BS = B // NCORES  # 16
LT = L // 128     # 8 L-tiles
PT = P // 128     # 4 P-tiles

_cache = {}


def _build_nc():
    import concourse.bass as bass
    import concourse.bacc as bacc
    import concourse.bass_isa as bass_isa
    import concourse.tile as tile
    from concourse import mybir

    f32 = mybir.dt.float32
    bf16 = mybir.dt.bfloat16
    fp8 = mybir.dt.float8e4
    u32 = mybir.dt.uint32
    Alu = mybir.AluOpType
    Act = mybir.ActivationFunctionType
    Ax = mybir.AxisListType

    nc = bacc.Bacc("TRN2", target_bir_lowering=False, debug=False)

    # ---- DRAM I/O (host pre-swizzles everything partition-major) ----
    xtb_d = nc.dram_tensor("xtb", (128, L), bf16, kind="ExternalInput")
    pct_d = nc.dram_tensor("pct", (128, LT * P), fp8, kind="ExternalInput")
    pcl_d = nc.dram_tensor("pcl", (128, PT * L), fp8, kind="ExternalInput")
    # bigf: [xl 0:128 | ec 128:144 | wt 144:152 | wr 152:156 | idn 156:284]
    NF = 284
    big_d = nc.dram_tensor("big", (128, NF), f32, kind="ExternalInput")
    y_d = nc.dram_tensor("y", (BS, L), f32, kind="ExternalOutput")

    NW = 9                    # wb0 wc0 wb1 wc1 wb2 wc2 wb3 wc3 | ones
    NB = 128 + BS             # m^T | m^T_loc
    NR = NB + NW              # combined-matmul rhs width
    HLT = LT // 2

    def bcast(col_ap, n):
        """Free-dim step-0 broadcast of a (...,1) AP to (...,n)."""
        return bass.AP(tensor=col_ap.tensor, offset=col_ap.offset,
                       ap=[*col_ap.ap[:-1], [0, n]])

    with tile.TileContext(nc) as tc:
        with (
            tc.tile_pool(name="sb", bufs=1) as sb,
            tc.tile_pool(name="ps", bufs=3, space="PSUM") as ps,
            tc.tile_pool(name="ps2", bufs=2, space="PSUM") as ps2,
            tc.tile_pool(name="ps4", bufs=1, space="PSUM") as ps4,
            tc.tile_pool(name="ps3", bufs=1, space="PSUM") as ps3,
        ):
            XTB = sb.tile([128, LT, 128], bf16)
            PCT = sb.tile([128, LT, P], fp8)
            PCL = sb.tile([128, PT, L], fp8)
            BIG = sb.tile([128, NF], f32)
            XL = BIG[:, 0:128].rearrange("p (i j) -> p i j", i=LT)
            EC = BIG[:, 128:144]
            WT = BIG[:, 144:152].rearrange("p (k t) -> p k t", k=PT)
            WRp = BIG[:, 152:156]
            IDN = BIG[:, 156:284]
            # One HWDGE ring; FIFO order = transfer priority.
            nc.sync.dma_start(XTB[:, 0:4, :].rearrange("p i b -> p (i b)"),
                              xtb_d[:, 0:512])
            nc.sync.dma_start(PCT[:, 0:4, :].rearrange("p i q -> p (i q)"),
                              pct_d[:, 0:4 * P])
            nc.sync.dma_start(XTB[:, 4:8, :].rearrange("p i b -> p (i b)"),
                              xtb_d[:, 512:1024])
            nc.sync.dma_start(PCT[:, 4:8, :].rearrange("p i q -> p (i q)"),
                              pct_d[:, 4 * P:8 * P])
            nc.sync.dma_start(BIG[:], big_d[:])
            nc.sync.dma_start(PCL[:].rearrange("p k l -> p (k l)"), pcl_d[:])

            # ---- hard decisions (transposed, fp8 {0,1}) ----
            HT = sb.tile([128, LT, 128], fp8)
            for h in range(2):
                nc.vector.tensor_scalar(
                    HT[:, 4 * h:4 * h + 4, :].rearrange("p i b -> p (i b)"),
                    XTB[:, 4 * h:4 * h + 4, :].rearrange("p i b -> p (i b)"),
                    0.0, None, Alu.is_lt)

            # ---- syndrome counts: S[b,q] = sum_l hard[b,l]*pc[q,l] ----
            S_ps = ps.tile([128, P], f32, tag="mm")
            for g in range(LT // 2):
                nc.tensor.matmul(S_ps[:], HT[:, 2 * g:2 * g + 2, :],
                                 PCT[:, 2 * g:2 * g + 2, :],
                                 perf_mode=mybir.MatmulPerfMode.DoubleRow,
                                 start=(g == 0), stop=(g == LT // 2 - 1))

            # ---- combined rhs (fp8): [ m^T | m^T_loc | W 4-term splits | ones ]
            # Wb/Wc are carried as 4 scaled fp8 terms each: w = sum_k t_k/16^k,
            # with t_k stored as fp8(residual_k * 16^k) so terms stay in
            # fp8's normal range.  Reconstruction happens after the matmul.
            RHS = sb.tile([128, PT, NR], fp8)
            R1 = sb.tile([128, PT, 2], f32)
            R2 = sb.tile([128, PT, 2], f32)
            R3 = sb.tile([128, PT, 2], f32)
            for k in range(PT):
                nc.scalar.copy(RHS[:, k, NB:NB + 2], WT[:, k, :])            # t0
                nc.vector.tensor_tensor(R1[:, k, :], WT[:, k, :],
                                        RHS[:, k, NB:NB + 2], Alu.subtract)
                nc.vector.tensor_scalar(RHS[:, k, NB + 2:NB + 4], R1[:, k, :],
                                        16.0, None, Alu.mult)                # t1
                nc.vector.scalar_tensor_tensor(R2[:, k, :],
                                               RHS[:, k, NB + 2:NB + 4],
                                               -1.0 / 16.0, R1[:, k, :],
                                               Alu.mult, Alu.add)
                nc.vector.tensor_scalar(RHS[:, k, NB + 4:NB + 6], R2[:, k, :],
                                        256.0, None, Alu.mult)               # t2
                nc.vector.scalar_tensor_tensor(R3[:, k, :],
                                               RHS[:, k, NB + 4:NB + 6],
                                               -1.0 / 256.0, R2[:, k, :],
                                               Alu.mult, Alu.add)
                nc.vector.tensor_scalar(RHS[:, k, NB + 6:NB + 8], R3[:, k, :],
                                        4096.0, None, Alu.mult)              # t3
                nc.vector.memset(RHS[:, k, NB + 8:NB + 9], 1.0)              # ones
            # early scalar chain: 2*sum(Wr) on every partition (independent of M)
            wrs = sb.tile([128, 1], f32)
            nc.vector.reduce_sum(wrs[:], WRp, axis=Ax.X)
            wrsum = sb.tile([128, 1], f32)
            nc.gpsimd.partition_all_reduce(wrsum[:], wrs[:], 128,
                                           bass_isa.ReduceOp.add)
            SCs2 = sb.tile([128, 1], f32)
            nc.vector.tensor_scalar(SCs2[:], wrsum[:], 2.0, None, Alu.mult)

            # parity m = S mod 2 (exact integer bit trick), chunked for overlap
            mag = sb.tile([128, P], f32)
            magu = sb.tile([128, P], u32)
            m_f = sb.tile([128, P], f32)
            for k in range(PT):
                ck = slice(k * 128, (k + 1) * 128)
                nc.vector.tensor_scalar(mag[:, ck], S_ps[:, ck], float(2 ** 23),
                                        None, Alu.add)
                nc.vector.tensor_scalar(magu[:, ck], mag[:, ck].bitcast(u32), 1,
                                        None, Alu.bitwise_and)
                nc.vector.tensor_copy(m_f[:, ck], magu[:, ck])
                mt_ps = ps2.tile([128, 128], f32, tag="tp")
                nc.tensor.transpose(mt_ps[:], m_f[:, ck], IDN)
                nc.scalar.copy(RHS[:, k, 0:128], mt_ps[:])
                ml_ps = ps4.tile([128, BS], f32, tag="tp2")
                nc.tensor.matmul(ml_ps[:], m_f[:, ck], EC)
                nc.scalar.copy(RHS[:, k, 128:NB], ml_ps[:])
            cnt = sb.tile([128, 1], f32)
            nc.vector.reduce_sum(cnt[:], m_f[:], axis=Ax.X)
            # per-row scale: alpha = 2*sum(Wr)*has_err (local rows; early)
            cl_ps = ps4.tile([BS, 1], f32, tag="tp2")
            nc.tensor.matmul(cl_ps[:], EC, cnt[:])
            HE = sb.tile([BS, 1], f32)
            nc.vector.tensor_scalar(HE[:], cl_ps[:], 0.0, None, Alu.is_gt)
            AL = sb.tile([BS, 1], f32)
            nc.vector.tensor_tensor(AL[:], HE[:], SCs2[0:BS, 0:1], Alu.mult)

            # ---- combined matmul over P (fp8 DoubleRow):  OUT = pc^T @ RHS ----
            WBA = sb.tile([128, LT, NW], f32)    # raw W-term columns + n
            WBCN = sb.tile([128, LT, 2], f32)    # reconstructed wb, wc per l
            BBT = sb.tile([128, LT, NB], f32)    # Bbias^T: full batch | local
            AMX = sb.tile([128, LT], f32)
            for t in range(LT):
                out_ps = ps.tile([128, NR], f32, tag="mm")
                for g in range(PT // 2):
                    nc.tensor.matmul(out_ps[:],
                                     PCL[:, 2 * g:2 * g + 2, t * 128:(t + 1) * 128],
                                     RHS[:, 2 * g:2 * g + 2, :],
                                     perf_mode=mybir.MatmulPerfMode.DoubleRow,
                                     start=(g == 0), stop=(g == PT // 2 - 1))
                nc.scalar.copy(WBA[:, t, :], out_ps[:, NB:NB + NW])
                # Bbias^T = n - 2*G^T on the scalar engine (keeps DVE free)
                nc.scalar.activation(BBT[:, t, :], out_ps[:, 0:NB], Act.Identity,
                                     bias=WBA[:, t, 8:9], scale=-2.0)
                nc.vector.tensor_reduce(AMX[:, t:t + 1], BBT[:, t, 0:128], axis=Ax.X,
                                        op=Alu.max, apply_absolute_value=True)
                if t % 2 == 1:
                    # wb,wc = ((t3/16 + t2)/16 + t1)/16 + t0, one tile-pair at a time
                    pr = slice(t - 1, t + 1)
                    nc.vector.scalar_tensor_tensor(WBCN[:, pr, :], WBA[:, pr, 6:8],
                                                   1.0 / 16.0, WBA[:, pr, 4:6],
                                                   Alu.mult, Alu.add)
                    nc.vector.scalar_tensor_tensor(WBCN[:, pr, :], WBCN[:, pr, :],
                                                   1.0 / 16.0, WBA[:, pr, 2:4],
                                                   Alu.mult, Alu.add)
                    nc.vector.scalar_tensor_tensor(WBCN[:, pr, :], WBCN[:, pr, :],
                                                   1.0 / 16.0, WBA[:, pr, 0:2],
                                                   Alu.mult, Alu.add)

            # ---- global 1/M on every partition via gpsimd all-reduce ----
            AMXr = sb.tile([128, 1], f32)
            nc.vector.tensor_reduce(AMXr[:], AMX[:], axis=Ax.X, op=Alu.max)
            Mall = sb.tile([128, 1], f32)
            nc.gpsimd.partition_all_reduce(Mall[:], AMXr[:], 128,
                                           bass_isa.ReduceOp.max)
            SCs1 = sb.tile([128, 1], f32)
            nc.vector.reciprocal(SCs1[:], Mall[:])

            # ---- local elementwise (6+2 split: big part overlaps combined) ----
            XA = sb.tile([128, LT, BS], f32)
            A1 = sb.tile([128, LT, BS], f32)
            T1 = sb.tile([128, LT, BS], f32)
            A2 = sb.tile([128, LT, BS], f32)
            C2 = sb.tile([128, LT, BS], f32)
            U = sb.tile([128, LT, BS], f32)
            W = sb.tile([128, LT, BS], f32)
            Q8 = sb.tile([128, LT, BS], f32)
            qt_psA = ps3.tile([BS, 512], f32, tag="qta")
            qt_psB = ps3.tile([BS, 512], f32, tag="qtb")

            def flat(ap):
                return ap.rearrange("p i j -> p (i j)")

            # QS = n*C*(t1 + Bbias/M) = CN*(T1 + invM*BBL); only the last
            # two ops are gated on M.
            CN = U   # reuse tiles
            for s0 in range(0, LT, 4):
                hs = slice(s0, s0 + 4)
                nc.scalar.activation(flat(XA[:, hs, :]), flat(XL[:, hs, :]), Act.Abs)
                nc.vector.tensor_tensor(A1[:, hs, :], XA[:, hs, :],
                                        bcast(WBCN[:, hs, 0:1], BS), Alu.mult)
                nc.scalar.activation(flat(T1[:, hs, :]), flat(A1[:, hs, :]), Act.Tanh)
                nc.vector.tensor_tensor(A2[:, hs, :], XA[:, hs, :],
                                        bcast(WBCN[:, hs, 1:2], BS), Alu.mult)
                nc.scalar.activation(flat(C2[:, hs, :]), flat(A2[:, hs, :]), Act.Tanh)
                nc.vector.scalar_tensor_tensor(CN[:, hs, :], C2[:, hs, :], 0.5,
                                               bcast(WBA[:, hs, 8:9], BS),
                                               Alu.add, Alu.mult)
            UU = W
            QS = Q8
            nc.vector.scalar_tensor_tensor(UU[:], BBT[:, :, 128:NB], SCs1[:, 0:1],
                                           T1[:], Alu.mult, Alu.add)
            nc.vector.tensor_tensor(QS[:], CN[:], UU[:], Alu.mult)
            for j in range(LT):
                qp, jo = (qt_psA, j) if j < 4 else (qt_psB, j - 4)
                nc.tensor.transpose(qp[:, jo * 128:(jo + 1) * 128], QS[:, j, :], IDN)

            # ---- alpha & softmax: quarters pipelined across ACT/DVE ----
            QFa = sb.tile([BS, 512], f32)
            QFb = sb.tile([BS, 512], f32)
            nm4 = sb.tile([BS, 4], f32)
            for q in range(2):
                cq = slice(q * 256, (q + 1) * 256)
                nc.scalar.activation(QFa[:, cq], qt_psA[:, cq], Act.Copy,
                                     scale=AL[:, 0:1])
                nc.vector.tensor_reduce(nm4[:, q:q + 1], QFa[:, cq], axis=Ax.X,
                                        op=Alu.max, negate=True)
            for q in range(2):
                cq = slice(q * 256, (q + 1) * 256)
                nc.vector.tensor_scalar(QFb[:, cq], qt_psB[:, cq], AL[:, 0:1],
                                        None, Alu.mult)
                nc.vector.tensor_reduce(nm4[:, 2 + q:3 + q], QFb[:, cq], axis=Ax.X,
                                        op=Alu.max, negate=True)
            nmx = sb.tile([BS, 1], f32)
            nc.vector.tensor_reduce(nmx[:], nm4[:], axis=Ax.X, op=Alu.min)
            EXa = sb.tile([BS, 512], f32)
            EXb = sb.tile([BS, 512], f32)
            ssa = sb.tile([BS, 1], f32)
            ssb = sb.tile([BS, 1], f32)
            nc.scalar.activation(EXb[:], QFb[:], Act.Exp, bias=nmx[:, 0:1], scale=1.0,
                                 accum_out=ssb[:])
            nc.scalar.activation(EXa[:], QFa[:], Act.Exp, bias=nmx[:, 0:1], scale=1.0,
                                 accum_out=ssa[:])
            ssum = sb.tile([BS, 1], f32)
            nc.vector.tensor_tensor(ssum[:], ssa[:], ssb[:], Alu.add)
            rs = sb.tile([BS, 1], f32)
            nc.vector.reciprocal(rs[:], ssum[:])
            OUTa = sb.tile([BS, 512], f32)
            OUTb = sb.tile([BS, 512], f32)
            nc.vector.tensor_scalar(OUTb[:], EXb[:], rs[:, 0:1], None, Alu.mult)
            nc.scalar.activation(OUTa[:], EXa[:], Act.Copy, scale=rs[:, 0:1])
            nc.sync.dma_start(y_d[:, 512:1024], OUTb[:])
            nc.sync.dma_start(y_d[:, 0:512], OUTa[:])

    nc.compile()
    return nc


def _prep_in_maps(X, pc_matrix, Wb, Wc, Wr, br):
    bf16 = ml_dtypes.bfloat16
    fp8 = ml_dtypes.float8_e4m3
    X = np.ascontiguousarray(np.asarray(X, dtype=np.float32))
    pc = np.asarray(pc_matrix)
    xT = X[:, :, 0].T  # (L, B)

    xtb = np.ascontiguousarray(
        xT.astype(bf16).reshape(LT, 128, B).transpose(1, 0, 2).reshape(128, LT * B))
    pct = np.ascontiguousarray(
        pc.T.astype(fp8).reshape(LT, 128, P).transpose(1, 0, 2).reshape(128, LT * P))
    pcl = np.ascontiguousarray(
        pc.astype(fp8).reshape(PT, 128, L).transpose(1, 0, 2).reshape(128, PT * L))
    w3 = np.stack([np.asarray(Wb, dtype=np.float32)[0],
                   np.asarray(Wc, dtype=np.float32)[0]], axis=1)  # (P, 2)
    wt = w3.reshape(PT, 128, 2).transpose(1, 0, 2).reshape(128, PT * 2)
    wrp = np.asarray(Wr, dtype=np.float32).reshape(128, 4)
    idn = np.eye(128, dtype=np.float32)

    in_maps = []
    for c in range(NCORES):
        sel = slice(c * BS, (c + 1) * BS)
        ec = np.zeros((128, BS), dtype=np.float32)
        ec[np.arange(c * BS, (c + 1) * BS), np.arange(BS)] = 1.0
        xl = xT[:, sel].reshape(LT, 128, BS).transpose(1, 0, 2).reshape(128, LT * BS)
        big = np.concatenate([xl, ec, wt, wrp, idn], axis=1).astype(np.float32)
        assert big.shape == (128, 284)
        in_maps.append({"xtb": xtb, "pct": pct, "pcl": pcl,
                        "big": np.ascontiguousarray(big)})
    return in_maps


def run(inputs, trace=False, **kw):
    if "nc" not in _cache:
        _cache["nc"] = _build_nc()
    nc = _cache["nc"]
    in_maps = _prep_in_maps(**inputs)
    from concourse.bass_utils import run_bass_kernel_spmd
    res = run_bass_kernel_spmd(nc, in_maps, core_ids=list(range(NCORES)),
                               trace=trace, **kw)
    out = np.concatenate([res.results[c]["y"] for c in range(NCORES)], axis=0)
    return np.ascontiguousarray(out[:, :, None].astype(np.float32)), res


def kernel(**inputs) -> np.ndarray:
    out, _ = run(inputs)
    return out



# revision 4
# speedup vs baseline: 1.1959x; 1.1959x over previous
"""Trainium2 Bass kernel for nn_EncoderLayer_42399917146737.

The reference "SSM scan" is degenerate: at every step i the recurrence
overwrites h at exactly the positions p with pc[p,i]==1 with the scalar
b_i, and the step output reads only those positions.  Hence

    y_i[b] = C[b,i] * Bcoef[b,i] * n_i,      n_i = sum_p pc[p,i]

with no sequential dependence, and the reverse scan equals the forward
one.  The broadcast over p then reduces the Wr projection to a scalar
sum, so the whole module collapses to

    logits[b,l] = 2*sum(Wr) * has_err[b] * n_l * C[b,l] * (Bbias[b,l]/M + tanh(|X[b,l]|*wb_l))
    out         = softmax_l(logits)

where  Bbias = n_l - 2*G,  G = m @ pc,  m = parity(hard @ pc^T),
hard = (X<0),  M = max|Bbias| (GLOBAL over the full batch),
wb = Wb @ pc,  wc = Wc @ pc,  C = 0.5 + tanh(|X|*wc_l).
(br shifts all logits equally -> drops out of softmax.)

wb, wc, n_l and 2*sum(Wr) are X-independent weight folds -> host.

Sharding: batch B=128 over 8 cores (16 rows each).  M is a global max
over the whole batch, so every core recomputes the (cheap) full-batch
parity and Bbias matmuls; per-batch elementwise work + softmax run on
the core's own 16 rows.  The whole pipeline stays L-major (l on
partitions): the syndrome matmul is done transposed (S^T per 128-q
chunk) so parity writes the combined-matmul RHS directly from PSUM,
and the softmax row-reductions use gpsimd partition all-reduces.  The
(128p=l, 128=t*16+b) output tile is unswizzled on the host.

Precision: pc/hard/m are {0,1} so fp8 matmuls with f32 accumulate are
exact; X^T for sign tests rides in bf16 (sign-exact); everything else
is f32.
"""

import numpy as np
import ml_dtypes

B, L, P = 128, 1024, 512
NCORES = 8
BS = B // NCORES  # 16
LT = L // 128     # 8 L-tiles
PT = P // 128     # 4 P-(=q-)chunks
NB = 128 + BS     # rhs cols: full-batch m^T | local m^T

_cache = {}


def _build_nc():
    import concourse.bass as bass
    import concourse.bacc as bacc
    import concourse.bass_isa as bass_isa
    import concourse.tile as tile
    from concourse import mybir

    f32 = mybir.dt.float32
    bf16 = mybir.dt.bfloat16
    fp8 = mybir.dt.float8e4
    u32 = mybir.dt.uint32
    Alu = mybir.AluOpType
    Act = mybir.ActivationFunctionType
    Ax = mybir.AxisListType
    DR = mybir.MatmulPerfMode.DoubleRow

    nc = bacc.Bacc("TRN2", target_bir_lowering=False, debug=False)

    # ---- DRAM I/O (host pre-swizzles everything partition-major) ----
    xtb_d = nc.dram_tensor("xtb", (128, L), bf16, kind="ExternalInput")
    pct_d = nc.dram_tensor("pct", (128, LT * P), fp8, kind="ExternalInput")
    pcl_d = nc.dram_tensor("pcl", (128, LT * P), fp8, kind="ExternalInput")
    # big: [xl 0:128 | wbc 128:144 | nl 144:152 | wrs2 152]
    NF = 153
    big_d = nc.dram_tensor("big", (128, NF), f32, kind="ExternalInput")
    y_d = nc.dram_tensor("y", (128, LT * BS), f32, kind="ExternalOutput")

    def bmid(t2, n):
        """Broadcast a (128, m) AP to (128, n, m) with a step-0 middle dim."""
        return bass.AP(tensor=t2.tensor, offset=t2.offset,
                       ap=[t2.ap[0], [0, n], *t2.ap[1:]])

    def blast(col_ap, n):
        """Free-dim step-0 broadcast of a (...,1) AP to (...,n)."""
        return bass.AP(tensor=col_ap.tensor, offset=col_ap.offset,
                       ap=[*col_ap.ap[:-1], [0, n]])

    with tile.TileContext(nc) as tc:
        with (
            tc.tile_pool(name="sb", bufs=1) as sb,
            tc.tile_pool(name="psS", bufs=2, space="PSUM") as psS,
            tc.tile_pool(name="psC", bufs=3, space="PSUM") as psC,
            tc.tile_pool(name="psN", bufs=1, space="PSUM") as psN,
        ):
            XTB = sb.tile([128, LT, 128], bf16)
            PCT = sb.tile([128, LT, P], fp8)
            PCL = sb.tile([128, LT, PT, 128], fp8)
            BIG = sb.tile([128, NF], f32)
            XL = BIG[:, 0:128].rearrange("p (i j) -> p i j", i=LT)
            WBC = BIG[:, 128:144].rearrange("p (i c) -> p i c", i=LT)
            NL = BIG[:, 144:152].rearrange("p (i c) -> p i c", i=LT)
            WRS2 = BIG[:, 152:153]
            # Three DMA queues in parallel: SP (xtb+big), Act HWDGE (pct),
            # gpsimd SWDGE (pcl).
            nc.sync.dma_start(XTB[:, 0:4, :].rearrange("p i b -> p (i b)"),
                              xtb_d[:, 0:512])
            nc.scalar.dma_start(PCT[:].rearrange("p i q -> p (i q)"), pct_d[:])
            nc.gpsimd.dma_start(PCL[:].rearrange("p t k j -> p (t k j)"),
                                pcl_d[:])
            nc.sync.dma_start(XTB[:, 4:8, :].rearrange("p i b -> p (i b)"),
                              xtb_d[:, 512:1024])
            nc.sync.dma_start(BIG[:], big_d[:])

            # ---- hard decisions, transposed, fp8 {0,1}: [full | local] ----
            HTA = sb.tile([128, LT, NB], fp8)
            for h in range(2):
                nc.vector.tensor_scalar(HTA[:, 4 * h:4 * h + 4, 0:128],
                                        XTB[:, 4 * h:4 * h + 4, :],
                                        0.0, None, Alu.is_lt)
            nc.vector.tensor_scalar(HTA[:, :, 128:NB], XL[:], 0.0, None,
                                    Alu.is_lt)
            ONES8 = sb.tile([128, 1], fp8)
            nc.vector.memset(ONES8[:], 1.0)

            # ---- transposed syndrome + parity -> RHS (fp8 {0,1}) ----
            # S^T_k[q,b] = sum_l pc[k*128+q, l] * hard[b, l]; m = S mod 2.
            RHS = sb.tile([128, PT, NB], fp8)
            MAG = sb.tile([128, PT, NB], f32)
            cnt_ps = psN.tile([1, BS], f32, tag="cnt")
            for k in range(PT):
                st = psS.tile([128, NB], f32, tag="st")
                for g in range(4):
                    nc.tensor.matmul(st[:],
                                     PCT[:, 2 * g:2 * g + 2,
                                         k * 128:(k + 1) * 128],
                                     HTA[:, 2 * g:2 * g + 2, :],
                                     perf_mode=DR, start=(g == 0),
                                     stop=(g == 3))
                # parity: (S + 2^23) aligns the integer LSB with mantissa
                # bit 0; mask it and cast the {0,1} int to fp8.
                nc.vector.tensor_scalar(MAG[:, k, :], st[:], float(2 ** 23),
                                        None, Alu.add)
                nc.vector.tensor_scalar(MAG[:, k, :].bitcast(u32),
                                        MAG[:, k, :].bitcast(u32), 1,
                                        None, Alu.bitwise_and)
                nc.vector.tensor_copy(RHS[:, k, :], MAG[:, k, :].bitcast(u32))
                # local error counts: cnt[b] += sum_q m^T_loc[q, b]
                nc.tensor.matmul(cnt_ps[:], ONES8[:], RHS[:, k, 128:NB],
                                 start=(k == 0), stop=(k == PT - 1))

            # alpha row = 2*sum(Wr) * has_err for the local batch, then
            # broadcast down all partitions for the L-major tail.
            ALr = sb.tile([1, BS], f32)
            nc.vector.tensor_scalar(ALr[:], cnt_ps[:], 0.0, None, Alu.is_gt)
            nc.vector.tensor_tensor(ALr[:], ALr[:], blast(WRS2[0:1, 0:1], BS),
                                    Alu.mult)
            ALb = sb.tile([128, BS], f32)
            nc.gpsimd.partition_broadcast(ALb[:], ALr[:], 128)

            # ---- local elementwise (overlaps the DMAs / matmuls) ----
            XA = sb.tile([128, LT, BS], f32)
            A1 = sb.tile([128, LT, BS], f32)
            T1 = sb.tile([128, LT, BS], f32)
            C2 = sb.tile([128, LT, BS], f32)
            CN = sb.tile([128, LT, BS], f32)
            nc.scalar.activation(XA[:].rearrange("p i j -> p (i j)"),
                                 BIG[:, 0:128], Act.Abs)
            nc.vector.tensor_tensor(A1[:], XA[:], blast(WBC[:, :, 0:1], BS),
                                    Alu.mult)
            nc.scalar.activation(T1[:].rearrange("p i j -> p (i j)"),
                                 A1[:].rearrange("p i j -> p (i j)"), Act.Tanh)
            nc.vector.tensor_tensor(A1[:], XA[:], blast(WBC[:, :, 1:2], BS),
                                    Alu.mult)
            nc.scalar.activation(C2[:].rearrange("p i j -> p (i j)"),
                                 A1[:].rearrange("p i j -> p (i j)"), Act.Tanh)
            nc.vector.scalar_tensor_tensor(CN[:], C2[:], 0.5,
                                           blast(NL[:, :, 0:1], BS),
                                           Alu.add, Alu.mult)

            # ---- combined matmul: G^T per L-tile; Bbias^T = n - 2 G^T ----
            BBT = sb.tile([128, LT, NB], f32)
            AMX = sb.tile([128, LT], f32)
            for t in range(LT):
                cb = psC.tile([128, NB], f32, tag="cb")
                for j in range(2):
                    nc.tensor.matmul(cb[:], PCL[:, t, 2 * j:2 * j + 2, :],
                                     RHS[:, 2 * j:2 * j + 2, :],
                                     perf_mode=DR, start=(j == 0),
                                     stop=(j == 1))
                nc.scalar.activation(BBT[:, t, :], cb[:], Act.Identity,
                                     bias=NL[:, t, 0:1], scale=-2.0)
                nc.vector.tensor_reduce(AMX[:, t:t + 1], BBT[:, t, 0:128],
                                        axis=Ax.X, op=Alu.max,
                                        apply_absolute_value=True)

            # ---- global 1/M on every partition ----
            AMXr = sb.tile([128, 1], f32)
            nc.vector.tensor_reduce(AMXr[:], AMX[:], axis=Ax.X, op=Alu.max)
            Mall = sb.tile([128, 1], f32)
            nc.gpsimd.partition_all_reduce(Mall[:], AMXr[:], 128,
                                           bass_isa.ReduceOp.max)
            SC1 = sb.tile([128, 1], f32)
            nc.vector.reciprocal(SC1[:], Mall[:])

            # ---- logits, softmax over l (partitions x tiles), L-major ----
            UU = A1  # reuse
            QS = XA
            LG = sb.tile([128, LT, BS], f32)
            nc.vector.scalar_tensor_tensor(UU[:], BBT[:, :, 128:NB],
                                           SC1[:, 0:1], T1[:],
                                           Alu.mult, Alu.add)
            nc.vector.tensor_tensor(QS[:], CN[:], UU[:], Alu.mult)
            nc.vector.tensor_tensor(LG[:], QS[:], bmid(ALb[:], LT), Alu.mult)
            MXt = sb.tile([128, 4, BS], f32)
            nc.vector.tensor_tensor(MXt[:], LG[:, 0:4, :], LG[:, 4:8, :],
                                    Alu.max)
            nc.vector.tensor_tensor(MXt[:, 0:2, :], MXt[:, 0:2, :],
                                    MXt[:, 2:4, :], Alu.max)
            nc.vector.tensor_tensor(MXt[:, 0, :], MXt[:, 0, :], MXt[:, 1, :],
                                    Alu.max)
            MX = sb.tile([128, BS], f32)
            nc.gpsimd.partition_all_reduce(MX[:], MXt[:, 0, :], 128,
                                           bass_isa.ReduceOp.max)
            ES = C2  # reuse
            EX = sb.tile([128, LT, BS], f32)
            nc.vector.tensor_tensor(ES[:], LG[:], bmid(MX[:], LT),
                                    Alu.subtract)
            nc.scalar.activation(EX[:].rearrange("p i j -> p (i j)"),
                                 ES[:].rearrange("p i j -> p (i j)"), Act.Exp)
            ES4 = CN  # reuse
            nc.vector.tensor_tensor(ES4[:, 0:4, :], EX[:, 0:4, :],
                                    EX[:, 4:8, :], Alu.add)
            nc.vector.tensor_tensor(ES4[:, 0:2, :], ES4[:, 0:2, :],
                                    ES4[:, 2:4, :], Alu.add)
            nc.vector.tensor_tensor(ES4[:, 0, :], ES4[:, 0, :], ES4[:, 1, :],
                                    Alu.add)
            SS = sb.tile([128, BS], f32)
            nc.gpsimd.partition_all_reduce(SS[:], ES4[:, 0, :], 128,
                                           bass_isa.ReduceOp.add)
            RS = sb.tile([128, BS], f32)
            nc.vector.reciprocal(RS[:], SS[:])
            OUT = sb.tile([128, LT, BS], f32)
            nc.vector.tensor_tensor(OUT[:], EX[:], bmid(RS[:], LT), Alu.mult)
            nc.sync.dma_start(y_d[:], OUT[:].rearrange("p i j -> p (i j)"))

    nc.compile()
    return nc


def _prep_in_maps(X, pc_matrix, Wb, Wc, Wr, br):
    bf16 = ml_dtypes.bfloat16
    fp8 = ml_dtypes.float8_e4m3
    X = np.ascontiguousarray(np.asarray(X, dtype=np.float32))
    pc = np.asarray(pc_matrix).astype(np.float32)
    xT = X[:, :, 0].T  # (L, B)

    xtb = np.ascontiguousarray(
        xT.astype(bf16).reshape(LT, 128, B).transpose(1, 0, 2).reshape(128, LT * B))
    pct = np.ascontiguousarray(
        pc.T.astype(fp8).reshape(LT, 128, P).transpose(1, 0, 2).reshape(128, LT * P))
    pcl = np.ascontiguousarray(
        pc.astype(fp8).reshape(PT, 128, LT, 128).transpose(1, 2, 0, 3).reshape(128, LT * P))

    # host weight folds (X-independent)
    wb = (np.asarray(Wb, np.float64)[0] @ pc.astype(np.float64))  # (L,)
    wc = (np.asarray(Wc, np.float64)[0] @ pc.astype(np.float64))  # (L,)
    nl = pc.sum(axis=0)                                           # (L,)
    wrs2 = 2.0 * float(np.asarray(Wr, np.float64).sum())
    wbc = np.stack([wb, wc], axis=1).astype(np.float32)           # (L, 2)
    wbc = wbc.reshape(LT, 128, 2).transpose(1, 0, 2).reshape(128, LT * 2)
    nlm = nl.reshape(LT, 128).T.astype(np.float32)                # (128, LT)
    wr2 = np.full((128, 1), wrs2, dtype=np.float32)

    in_maps = []
    for c in range(NCORES):
        sel = slice(c * BS, (c + 1) * BS)
        xl = xT[:, sel].reshape(LT, 128, BS).transpose(1, 0, 2).reshape(128, LT * BS)
        big = np.concatenate([xl, wbc, nlm, wr2], axis=1).astype(np.float32)
        assert big.shape == (128, 153)
        in_maps.append({"xtb": xtb, "pct": pct, "pcl": pcl,
                        "big": np.ascontiguousarray(big)})
    return in_maps


def _unswizzle(y):
    """(128, LT*BS) L-major core output -> (BS, L)."""
    return y.reshape(128, LT, BS).transpose(2, 1, 0).reshape(BS, L)


def run(inputs, trace=False, **kw):
    if "nc" not in _cache:
        _cache["nc"] = _build_nc()
    nc = _cache["nc"]
    in_maps = _prep_in_maps(**inputs)
    from concourse.bass_utils import run_bass_kernel_spmd
    res = run_bass_kernel_spmd(nc, in_maps, core_ids=list(range(NCORES)),
                               trace=trace, **kw)
    out = np.concatenate([_unswizzle(res.results[c]["y"])
                          for c in range(NCORES)], axis=0)
    return np.ascontiguousarray(out[:, :, None].astype(np.float32)), res


def kernel(**inputs) -> np.ndarray:
    out, _ = run(inputs)
    return out


# revision 7
# speedup vs baseline: 1.3539x; 1.1321x over previous
"""Trainium2 Bass kernel for nn_EncoderLayer_42399917146737.

The reference "SSM scan" is degenerate: at every step i the recurrence
overwrites h at exactly the positions p with pc[p,i]==1 with the scalar
b_i, and the step output reads only those positions.  Hence

    y_i[b] = C[b,i] * Bcoef[b,i] * n_i,      n_i = sum_p pc[p,i]

with no sequential dependence, and the reverse scan equals the forward
one.  The broadcast over p then reduces the Wr projection to a scalar
sum, so the whole module collapses to

    logits[b,l] = 2*sum(Wr) * has_err[b] * n_l * C[b,l] * (Bbias[b,l]/M + tanh(|X[b,l]|*wb_l))
    out         = softmax_l(logits)

where  Bbias = n_l - 2*G,  G = m @ pc,  m = parity(hard @ pc^T),
hard = (X<0),  M = max|Bbias| (GLOBAL over the full batch),
wb = Wb @ pc,  wc = Wc @ pc,  C = 0.5 + tanh(|X|*wc_l).
(br shifts all logits equally -> drops out of softmax.)

wb, wc, n_l and 2*sum(Wr) are X-independent weight folds -> host.

Sharding: batch B=128 over 8 cores (16 rows each).  M is a global max
over the whole batch, so every core recomputes the (cheap) full-batch
parity and Bbias matmuls; per-batch elementwise work + softmax run on
the core's own 16 rows.  The whole pipeline stays L-major (l on
partitions): the syndrome matmul is done transposed (S^T per 128-q
chunk) so parity writes the combined-matmul RHS directly from PSUM,
and the softmax uses the online (two-level max) form so the
cross-partition max all-reduce overlaps the exp pass.  The per-core
(128p=l, b, t) output tile is unswizzled on the host.

Precision: pc/hard/m are {0,1} so fp8 matmuls with f32 accumulate are
exact; X^T for sign tests rides in bf16 (sign-exact); everything else
is f32.
"""

import numpy as np
import ml_dtypes

B, L, P = 128, 1024, 512
NCORES = 8
BS = B // NCORES  # 16
LT = L // 128     # 8 L-tiles
PT = P // 128     # 4 P-(=q-)chunks
NB = 128 + BS     # rhs cols: full-batch m^T | local m^T

_cache = {}


def _build_nc():
    import concourse.bass as bass
    import concourse.bacc as bacc
    import concourse.bass_isa as bass_isa
    import concourse.tile as tile
    from concourse import mybir

    f32 = mybir.dt.float32
    bf16 = mybir.dt.bfloat16
    fp8 = mybir.dt.float8e4
    u32 = mybir.dt.uint32
    Alu = mybir.AluOpType
    Act = mybir.ActivationFunctionType
    Ax = mybir.AxisListType
    DR = mybir.MatmulPerfMode.DoubleRow

    nc = bacc.Bacc("TRN2", target_bir_lowering=False, debug=False)

    # ---- DRAM I/O (host pre-swizzles everything partition-major) ----
    xtb_d = nc.dram_tensor("xtb", (128, L), bf16, kind="ExternalInput")
    pct_d = nc.dram_tensor("pct", (128, PT * L), fp8, kind="ExternalInput")
    pcl_d = nc.dram_tensor("pcl", (128, LT * P), fp8, kind="ExternalInput")
    # big: [xl 0:128 | wbc 128:144 | nl 144:152 | wrs2 152]
    NF = 153
    big_d = nc.dram_tensor("big", (128, NF), f32, kind="ExternalInput")
    y_d = nc.dram_tensor("y", (128, BS * LT), f32, kind="ExternalOutput")

    def bmid(t2, n):
        """Broadcast a (128, m) AP to (128, n, m) with a step-0 middle dim."""
        return bass.AP(tensor=t2.tensor, offset=t2.offset,
                       ap=[t2.ap[0], [0, n], *t2.ap[1:]])

    def blast(t2, n):
        """Broadcast a (128, m) AP to (128, m, n) with a step-0 last dim."""
        return bass.AP(tensor=t2.tensor, offset=t2.offset,
                       ap=[*t2.ap, [0, n]])

    with tile.TileContext(nc) as tc:
        with (
            tc.tile_pool(name="sb", bufs=1) as sb,
            tc.tile_pool(name="psS", bufs=2, space="PSUM") as psS,
            tc.tile_pool(name="psC", bufs=3, space="PSUM") as psC,
            tc.tile_pool(name="psN", bufs=1, space="PSUM") as psN,
        ):
            XTB = sb.tile([128, LT, 128], bf16)
            PCT = sb.tile([128, PT, LT, 128], fp8)
            PCL = sb.tile([128, LT, PT, 128], fp8)
            BIG = sb.tile([128, NF], f32)
            XL = BIG[:, 0:128].rearrange("p (j i) -> p j i", j=BS)
            WBC = BIG[:, 128:144].rearrange("p (c i) -> p c i", c=2)
            NL = BIG[:, 144:152]                      # (128, LT)
            WRS2 = BIG[:, 152:153]
            # SP queue: big + xtb; Act HWDGE queue: pct then pcl, each in
            # two chunks so dependents start on the first half.
            nc.sync.dma_start(BIG[:], big_d[:])
            nc.sync.dma_start(XTB[:, 0:4, :].rearrange("p i b -> p (i b)"),
                              xtb_d[:, 0:512])
            nc.sync.dma_start(XTB[:, 4:8, :].rearrange("p i b -> p (i b)"),
                              xtb_d[:, 512:1024])
            nc.scalar.dma_start(
                PCT[:, 0:2].rearrange("p k i j -> p (k i j)"),
                pct_d[:, 0:2 * L])
            nc.scalar.dma_start(
                PCT[:, 2:4].rearrange("p k i j -> p (k i j)"),
                pct_d[:, 2 * L:4 * L])
            nc.scalar.dma_start(
                PCL[:, 0:4].rearrange("p t k j -> p (t k j)"),
                pcl_d[:, 0:4 * P])
            nc.scalar.dma_start(
                PCL[:, 4:8].rearrange("p t k j -> p (t k j)"),
                pcl_d[:, 4 * P:8 * P])

            # ---- hard decisions, transposed, fp8 {0,1}: [full | local] ----
            HTA = sb.tile([128, LT, NB], fp8)
            for h in range(2):
                nc.vector.tensor_scalar(HTA[:, 4 * h:4 * h + 4, 0:128],
                                        XTB[:, 4 * h:4 * h + 4, :],
                                        0.0, None, Alu.is_lt)
            # local hard bits from XL (b,t layout) scattered into (t,b) cols
            HTL = bass.AP(tensor=HTA.tensor, offset=HTA.offset + 128,
                          ap=[HTA.ap[0], [1, BS], [NB, LT]])
            nc.vector.tensor_scalar(HTL, XL[:], 0.0, None, Alu.is_lt)
            ONES8 = sb.tile([128, 1], fp8)
            nc.vector.memset(ONES8[:], 1.0)

            # ---- transposed syndrome + parity -> RHS (fp8 {0,1}) ----
            # S^T_k[q,b] = sum_l pc[k*128+q, l] * hard[b, l]; m = S mod 2.
            RHS = sb.tile([128, PT, NB], fp8)
            MAG = sb.tile([128, PT, NB], f32)
            cnt_ps = psN.tile([1, BS], f32, tag="cnt")
            for k in range(PT):
                st = psS.tile([128, NB], f32, tag="st")
                for g in range(4):
                    nc.tensor.matmul(st[:], PCT[:, k, 2 * g:2 * g + 2, :],
                                     HTA[:, 2 * g:2 * g + 2, :],
                                     perf_mode=DR, start=(g == 0),
                                     stop=(g == 3))
                # parity: (S + 2^23) aligns the integer LSB with mantissa
                # bit 0; mask it and cast the {0,1} int to fp8.
                nc.vector.tensor_scalar(MAG[:, k, :], st[:], float(2 ** 23),
                                        None, Alu.add)
                nc.vector.tensor_scalar(MAG[:, k, :].bitcast(u32),
                                        MAG[:, k, :].bitcast(u32), 1,
                                        None, Alu.bitwise_and)
                nc.vector.tensor_copy(RHS[:, k, :], MAG[:, k, :].bitcast(u32))
                # local error counts: cnt[b] += sum_q m^T_loc[q, b]
                nc.tensor.matmul(cnt_ps[:], ONES8[:], RHS[:, k, 128:NB],
                                 start=(k == 0), stop=(k == PT - 1))

            # alpha row = 2*sum(Wr) * has_err for the local batch, then
            # broadcast down all partitions for the L-major tail.
            ALr = sb.tile([1, BS], f32)
            nc.vector.tensor_scalar(ALr[:], cnt_ps[:], 0.0, None, Alu.is_gt)
            nc.vector.tensor_tensor(ALr[:], ALr[:], blast(WRS2[0:1, 0:1], BS),
                                    Alu.mult)
            ALb = sb.tile([128, BS], f32)
            nc.gpsimd.partition_broadcast(ALb[:], ALr[:], 128)

            # ---- local elementwise, (128, b, t) layout ----
            XA = sb.tile([128, BS, LT], f32)
            A1 = sb.tile([128, BS, LT], f32)
            T1 = sb.tile([128, BS, LT], f32)
            C2 = sb.tile([128, BS, LT], f32)
            CN = sb.tile([128, BS, LT], f32)
            ACb = sb.tile([128, BS, LT], f32)
            LG0 = sb.tile([128, BS, LT], f32)
            nc.scalar.activation(XA[:].rearrange("p j i -> p (j i)"),
                                 BIG[:, 0:128], Act.Abs)
            nc.vector.tensor_tensor(A1[:], XA[:], bmid(WBC[:, 0, :], BS),
                                    Alu.mult)
            nc.scalar.activation(T1[:].rearrange("p j i -> p (j i)"),
                                 A1[:].rearrange("p j i -> p (j i)"), Act.Tanh)
            nc.vector.tensor_tensor(A1[:], XA[:], bmid(WBC[:, 1, :], BS),
                                    Alu.mult)
            nc.scalar.activation(C2[:].rearrange("p j i -> p (j i)"),
                                 A1[:].rearrange("p j i -> p (j i)"), Act.Tanh)
            nc.vector.scalar_tensor_tensor(CN[:], C2[:], 0.5, bmid(NL[:], BS),
                                           Alu.add, Alu.mult)
            nc.vector.tensor_tensor(ACb[:], CN[:], blast(ALb[:], LT), Alu.mult)
            nc.vector.tensor_tensor(LG0[:], ACb[:], T1[:], Alu.mult)

            # ---- combined matmul: G^T per L-tile; Bbias^T = n - 2 G^T ----
            BBT = sb.tile([128, LT, NB], f32)
            AMX = sb.tile([128, LT], f32)
            for t in range(LT):
                cb = psC.tile([128, NB], f32, tag="cb")
                for j in range(2):
                    nc.tensor.matmul(cb[:], PCL[:, t, 2 * j:2 * j + 2, :],
                                     RHS[:, 2 * j:2 * j + 2, :],
                                     perf_mode=DR, start=(j == 0),
                                     stop=(j == 1))
                nc.scalar.activation(BBT[:, t, :], cb[:], Act.Identity,
                                     bias=NL[:, t:t + 1], scale=-2.0)
                nc.vector.tensor_reduce(AMX[:, t:t + 1], BBT[:, t, 0:128],
                                        axis=Ax.X, op=Alu.max,
                                        apply_absolute_value=True)

            # ---- global 1/M on every partition ----
            AMXr = sb.tile([128, 1], f32)
            nc.vector.tensor_reduce(AMXr[:], AMX[:], axis=Ax.X, op=Alu.max)
            Mall = sb.tile([128, 1], f32)
            nc.gpsimd.partition_all_reduce(Mall[:], AMXr[:], 128,
                                           bass_isa.ReduceOp.max)
            SC1 = sb.tile([128, 1], f32)
            nc.vector.reciprocal(SC1[:], Mall[:])

            # ---- logits + online softmax over l, all L-major ----
            # BBL: gather the local Bbias columns into (128, b, t)
            BBL = A1  # reuse
            BBsrc = bass.AP(tensor=BBT.tensor, offset=BBT.offset + 128,
                            ap=[BBT.ap[0], [1, BS], [NB, LT]])
            nc.vector.tensor_copy(BBL[:], BBsrc)
            LG1 = C2  # reuse
            nc.vector.tensor_tensor(LG1[:], ACb[:], BBL[:], Alu.mult)
            LG = CN  # reuse
            nc.vector.scalar_tensor_tensor(LG[:], LG1[:], SC1[:, 0:1],
                                           LG0[:], Alu.mult, Alu.add)
            MP = sb.tile([128, BS], f32)
            nc.vector.tensor_reduce(MP[:], LG[:], axis=Ax.X, op=Alu.max)
            MXg = sb.tile([128, BS], f32)
            nc.gpsimd.partition_all_reduce(MXg[:], MP[:], 128,
                                           bass_isa.ReduceOp.max)
            # overlaps the all-reduce: exp(LG - MP) and its per-row sums
            ES = XA  # reuse
            EX = LG0  # reuse
            nc.vector.tensor_tensor(ES[:], LG[:], blast(MP[:], LT),
                                    Alu.subtract)
            nc.scalar.activation(EX[:].rearrange("p j i -> p (j i)"),
                                 ES[:].rearrange("p j i -> p (j i)"), Act.Exp)
            SP_ = sb.tile([128, BS], f32)
            nc.vector.tensor_reduce(SP_[:], EX[:], axis=Ax.X, op=Alu.add)
            # correction exp(MP - MXg), partial sums, global sum
            CE = sb.tile([128, BS], f32)
            nc.vector.tensor_tensor(CE[:], MP[:], MXg[:], Alu.subtract)
            CEx = sb.tile([128, BS], f32)
            nc.scalar.activation(CEx[:], CE[:], Act.Exp)
            SC = sb.tile([128, BS], f32)
            nc.vector.tensor_tensor(SC[:], SP_[:], CEx[:], Alu.mult)
            SS = sb.tile([128, BS], f32)
            nc.gpsimd.partition_all_reduce(SS[:], SC[:], 128,
                                           bass_isa.ReduceOp.add)
            RS = sb.tile([128, BS], f32)
            nc.vector.reciprocal(RS[:], SS[:])
            Fc = sb.tile([128, BS], f32)
            nc.vector.tensor_tensor(Fc[:], CEx[:], RS[:], Alu.mult)
            OUT = sb.tile([128, BS, LT], f32)
            nc.vector.tensor_tensor(OUT[:], EX[:], blast(Fc[:], LT), Alu.mult)
            nc.sync.dma_start(y_d[:], OUT[:].rearrange("p j i -> p (j i)"))

    nc.compile()
    return nc


def _prep_in_maps(X, pc_matrix, Wb, Wc, Wr, br):
    bf16 = ml_dtypes.bfloat16
    fp8 = ml_dtypes.float8_e4m3
    X = np.ascontiguousarray(np.asarray(X, dtype=np.float32))
    pc = np.asarray(pc_matrix).astype(np.float32)
    xT = X[:, :, 0].T  # (L, B)

    xtb = np.ascontiguousarray(
        xT.astype(bf16).reshape(LT, 128, B).transpose(1, 0, 2).reshape(128, LT * B))
    # pct[p, k, g, j] = pc[k*128+j, g*128+p]
    pct = np.ascontiguousarray(
        pc.astype(fp8).reshape(PT, 128, LT, 128).transpose(3, 0, 2, 1).reshape(128, PT * L))
    # pcl[p, t, k, j] = pc[k*128+p, t*128+j]
    pcl = np.ascontiguousarray(
        pc.astype(fp8).reshape(PT, 128, LT, 128).transpose(1, 2, 0, 3).reshape(128, LT * P))

    # host weight folds (X-independent)
    wb = (np.asarray(Wb, np.float64)[0] @ pc.astype(np.float64))  # (L,)
    wc = (np.asarray(Wc, np.float64)[0] @ pc.astype(np.float64))  # (L,)
    nl = pc.sum(axis=0)                                           # (L,)
    wrs2 = 2.0 * float(np.asarray(Wr, np.float64).sum())
    # wbc[p, c, i]: c in {wb, wc}, i = L-tile
    wbc = np.stack([wb.reshape(LT, 128).T, wc.reshape(LT, 128).T],
                   axis=1).astype(np.float32).reshape(128, 2 * LT)
    nlm = nl.reshape(LT, 128).T.astype(np.float32)                # (128, LT)
    wr2 = np.full((128, 1), wrs2, dtype=np.float32)

    in_maps = []
    for c in range(NCORES):
        sel = slice(c * BS, (c + 1) * BS)
        # xl[p, j, i] = x[i*128+p, local j]
        xl = xT[:, sel].reshape(LT, 128, BS).transpose(1, 2, 0).reshape(128, BS * LT)
        big = np.concatenate([xl, wbc, nlm, wr2], axis=1).astype(np.float32)
        assert big.shape == (128, 153)
        in_maps.append({"xtb": xtb, "pct": pct, "pcl": pcl,
                        "big": np.ascontiguousarray(big)})
    return in_maps


def _unswizzle(y):
    """(128, BS*LT) L-major core output -> (BS, L)."""
    return y.reshape(128, BS, LT).transpose(1, 2, 0).reshape(BS, L)


def run(inputs, trace=False, **kw):
    if "nc" not in _cache:
        _cache["nc"] = _build_nc()
    nc = _cache["nc"]
    in_maps = _prep_in_maps(**inputs)
    from concourse.bass_utils import run_bass_kernel_spmd
    res = run_bass_kernel_spmd(nc, in_maps, core_ids=list(range(NCORES)),
                               trace=trace, **kw)
    out = np.concatenate([_unswizzle(res.results[c]["y"])
                          for c in range(NCORES)], axis=0)
    return np.ascontiguousarray(out[:, :, None].astype(np.float32)), res


def kernel(**inputs) -> np.ndarray:
    out, _ = run(inputs)
    return out
